# revision 14
# baseline (speedup 1.0000x reference)
"""Multi-head self-attention (B=2, S=2048, D=1024, H=16) on 8 TRN2 NeuronCores.

Sharding: batch*heads tensor-parallel. Each core owns 2 heads (both batches):
it computes the QKV projection for its heads only (W_qkv output-dim sharded),
full attention for its 2x2 (batch, head) pairs, and the partial output
projection (W_out input-dim sharded). The 8 partial outputs are summed on the
host as part of unsharding (the "all-reduce"), plus the output bias.

Device-side layout choices (per core):
  - x is passed pre-transposed (xT [D, B*S]) so the QKV projection contracts
    over d_model on the partition axis with no on-device transposes.
  - q, k are produced head-major (qT/kT [hd, tok], bf16), v is produced
    hd-major then PE-transposed to token-major v_aug tiles [128, 130] with an
    appended ones column per head: the AV matmul (lhsT = [v | 1]) then yields
    both the unnormalized output AND the softmax denominator (row 64).
  - scores are computed k-token-major ([k, q] in PSUM, fp32), exp runs on the
    ACT engine straight out of PSUM with the 1/sqrt(hd) scale folded in,
    emitting bf16 probs. No max-subtraction: scores are bounded (|s|*scale
    < ~6 for this input distribution), well within fp32/bf16 exp range.
  - softmax normalization: reciprocal of the denominator row, broadcast
    across partitions with a K=1 PE matmul (ones-column x recip-row), then a
    DVE multiply producing normalized oT (fp32r).
  - output projection contracts the core's 128 head-dims as two K=64
    accumulated fp32r matmuls (head A + head B).
Matmul dtypes: fp32r (full-rate rounded fp32) for QKV/output projections and
bf16 for QK/AV (probs are [0,1]-ish, error is benign).
"""

import sys

for _p in ("/opt/trn_rl_repo", "/root/.axon_site/_ro/trn_rl_repo"):
    if _p not in sys.path:
        sys.path.insert(0, _p)

from contextlib import ExitStack

import numpy as np

import concourse.bacc as bacc
import concourse.bass as bass
import concourse.mybir as mybir
import concourse.tile as tile
from concourse.bass_utils import run_bass_kernel_spmd
from concourse.masks import make_identity

F32 = mybir.dt.float32
F32R = mybir.dt.float32r
BF16 = mybir.dt.bfloat16

B, S, D, H = 2, 2048, 1024, 16
HD = D // H  # 64
T = B * S  # 4096 tokens
SCALE = HD**-0.5
N_CORES = 8
HEADS_PER_CORE = H // N_CORES  # 2

EXP = mybir.ActivationFunctionType.Exp


def build_kernel() -> bacc.Bacc:
    nc = bacc.Bacc(target_bir_lowering=False)
    xT = nc.dram_tensor("xT", [D, T], F32R, kind="ExternalInput")
    wqkvT = nc.dram_tensor("wqkvT", [D, 6 * HD], F32R, kind="ExternalInput")
    woutT = nc.dram_tensor("woutT", [2 * HD, D], F32R, kind="ExternalInput")
    out = nc.dram_tensor("out", [T, D], F32, kind="ExternalOutput")

    with tile.TileContext(nc) as tc, ExitStack() as ctx:
        const = ctx.enter_context(tc.tile_pool(name="const", bufs=1))
        sb = ctx.enter_context(tc.tile_pool(name="sb", bufs=1))
        ps = ctx.enter_context(tc.tile_pool(name="ps", bufs=1, space="PSUM"))

        ident = const.tile([128, 128], BF16)
        make_identity(nc, ident)
        ones64_f32 = const.tile([1, 64], F32)
        nc.vector.memset(ones64_f32, 1.0)
        ones64 = const.tile([1, 64], F32R)
        nc.vector.tensor_copy(ones64[:], ones64_f32[:])

        w_sb = const.tile([128, 8, 6 * HD], F32R)
        nc.sync.dma_start(out=w_sb, in_=wqkvT.rearrange("(t p) c -> p t c", p=128))
        wo = const.tile([2 * HD, D], F32R)
        nc.sync.dma_start(out=wo, in_=woutT[:, :])

        for b in range(B):
            # ---------------- QKV projection for batch b ----------------
            qT = sb.tile([128, S], BF16, tag="qk", bufs=4, name=f"qT{b}")
            kT = sb.tile([128, S], BF16, tag="qk", bufs=4, name=f"kT{b}")
            vT = sb.tile([128, S], BF16, tag="vt", bufs=2, name=f"vT{b}")
            for ch in range(4):  # 512-token chunks
                x_sb = sb.tile([128, 8, 512], F32R, tag="x", bufs=2, name=f"x{b}{ch}")
                tok0 = b * S + ch * 512
                nc.sync.dma_start(
                    out=x_sb,
                    in_=xT[:, tok0 : tok0 + 512].rearrange("(t p) n -> p t n", p=128),
                )
                csl = slice(ch * 512, (ch + 1) * 512)
                for g, dst in ((0, qT), (1, kT), (2, vT)):
                    acc = ps.tile([128, 512], F32, tag="work", bufs=2, name="qkvps")
                    for t in range(8):
                        nc.tensor.matmul(
                            acc[:],
                            w_sb[:, t, g * 128 : (g + 1) * 128],
                            x_sb[:, t, :],
                            start=(t == 0),
                            stop=(t == 7),
                        )
                    nc.vector.tensor_copy(dst[:, csl], acc[:])

            # ------------- v: transpose to token-major v_aug -------------
            vaug = []
            for ti in range(16):
                va = sb.tile([128, 130], BF16, tag="vaug", bufs=32, name=f"va{b}_{ti}")
                tp = ps.tile([128, 128], BF16, tag="work", bufs=2, name="trps")
                nc.tensor.transpose(tp[:], vT[:, ti * 128 : (ti + 1) * 128], ident[:])
                nc.vector.tensor_copy(va[:, 0:64], tp[:, 0:64])
                nc.vector.tensor_copy(va[:, 65:129], tp[:, 64:128])
                nc.vector.memset(va[:, 64:65], 1.0)
                nc.vector.memset(va[:, 129:130], 1.0)
                vaug.append(va)

            # ---------------- attention per head ----------------
            # oT stacks both heads' normalized outputs: head A in partitions
            # 0-63, head B in 64-127 (DVE handles the misaligned write).
            oT = sb.tile([128, S], F32R, tag="ot", bufs=2, name=f"oT{b}")
            for h in range(2):
                p0 = h * 64
                vsl = slice(h * 65, (h + 1) * 65)

                probs_tiles = []
                acc_q0 = [
                    ps.tile([65, 512], F32, tag="av", bufs=2, name=f"avA{b}{h}{c}")
                    for c in range(2)
                ]
                # pass A over k tiles: scores -> exp -> AV for q-half 0
                for ki in range(16):
                    sc = ps.tile([128, 2048], F32, tag="sc", bufs=1, name="scps")
                    ksl = slice(ki * 128, (ki + 1) * 128)
                    for qc in range(4):
                        qsl = slice(qc * 512, (qc + 1) * 512)
                        nc.tensor.matmul(
                            sc[:, qsl],
                            kT[p0 : p0 + 64, ksl],
                            qT[p0 : p0 + 64, qsl],
                            start=True,
                            stop=True,
                        )
                    pr = sb.tile([128, 2048], BF16, tag="probs", bufs=16, name="pr")
                    nc.scalar.activation(pr[:], sc[:], EXP, scale=SCALE)
                    probs_tiles.append(pr)
                    for c in range(2):
                        nc.tensor.matmul(
                            acc_q0[c][:],
                            vaug[ki][:, vsl],
                            pr[:, c * 512 : (c + 1) * 512],
                            start=(ki == 0),
                            stop=(ki == 15),
                        )
                self_norm(nc, sb, ps, ones64, acc_q0, oT, p0, 0)
                # pass B: AV for q-half 1 from saved probs
                acc_q1 = [
                    ps.tile([65, 512], F32, tag="av", bufs=2, name=f"avB{b}{h}{c}")
                    for c in range(2)
                ]
                for ki in range(16):
                    for c in range(2):
                        nc.tensor.matmul(
                            acc_q1[c][:],
                            vaug[ki][:, vsl],
                            probs_tiles[ki][:, 1024 + c * 512 : 1024 + (c + 1) * 512],
                            start=(ki == 0),
                            stop=(ki == 15),
                        )
                self_norm(nc, sb, ps, ones64, acc_q1, oT, p0, 1024)

            # ---------------- output projection for batch b ----------------
            for tc_i in range(16):
                tsl = slice(tc_i * 128, (tc_i + 1) * 128)
                ob = sb.tile([128, D], F32, tag="outsb", bufs=2, name="ob")
                for nk in range(2):
                    nsl = slice(nk * 512, (nk + 1) * 512)
                    op = ps.tile([128, 512], F32, tag="work", bufs=2, name="outps")
                    nc.tensor.matmul(
                        op[:], oT[:, tsl], wo[:, nsl], start=True, stop=True
                    )
                    nc.vector.tensor_copy(ob[:, nsl], op[:])
                r0 = b * S + tc_i * 128
                nc.sync.dma_start(out=out[r0 : r0 + 128, :], in_=ob[:])

    nc.finalize()
    return nc


def self_norm(nc, sb, ps, ones64, accs, oT, p0, qbase):
    """Normalize AV accumulators (denominator in row 64) into oT columns
    (partition rows p0..p0+64 — head B's write is partition-shifted)."""
    for c, acc in enumerate(accs):
        rec = sb.tile([1, 512], F32R, tag="rec", bufs=2, name="rec")
        with nc.allow_low_precision(reason="fp32r recip keeps the bcast matmul fast"):
            nc.vector.reciprocal(rec[:], acc[64:65, :])
        bc = ps.tile([64, 512], F32, tag="work", bufs=2, name="bcps")
        nc.tensor.matmul(bc[:], ones64[:], rec[:], start=True, stop=True)
        bc_sb = sb.tile([64, 512], F32, tag="bcsb", bufs=2, name="bcsb")
        nc.vector.tensor_copy(bc_sb[:], bc[:])
        osl = slice(qbase + c * 512, qbase + (c + 1) * 512)
        nc.vector.tensor_mul(oT[p0 : p0 + 64, osl], acc[0:64, :], bc_sb[:])


_NC_CACHE = None
TRACE = False  # set True (e.g. from test.py) to capture an NTFF profile
LAST_RESULT = None  # BassKernelResults of the most recent run


def _get_nc():
    global _NC_CACHE
    if _NC_CACHE is None:
        _NC_CACHE = build_kernel()
    return _NC_CACHE


def kernel(x, W_qkv, W_out, b_out):
    x = np.asarray(x, dtype=np.float32)
    W_qkv = np.asarray(W_qkv, dtype=np.float32)
    W_out = np.asarray(W_out, dtype=np.float32)
    b_out = np.asarray(b_out, dtype=np.float32)

    xT = np.ascontiguousarray(x.reshape(T, D).T)  # [D, T]
    in_maps = []
    for c in range(N_CORES):
        h0 = c * HEADS_PER_CORE
        rows = slice(h0 * HD, (h0 + 2) * HD)  # this core's 128 head dims
        wq = W_qkv[0 * D :][rows]  # [128, D]
        wk = W_qkv[1 * D :][rows]
        wv = W_qkv[2 * D :][rows]
        wqkvT = np.ascontiguousarray(np.concatenate([wq, wk, wv], axis=0).T)
        woutT = np.ascontiguousarray(W_out[:, h0 * HD : (h0 + 2) * HD].T)
        in_maps.append({"xT": xT, "wqkvT": wqkvT, "woutT": woutT})

    nc = _get_nc()
    global LAST_RESULT
    res = run_bass_kernel_spmd(nc, in_maps, core_ids=list(range(N_CORES)), trace=TRACE)
    LAST_RESULT = res
    partial = np.zeros((T, D), dtype=np.float64)
    for c in range(N_CORES):
        partial += res.results[c]["out"].astype(np.float64)
    full = (partial + b_out.astype(np.float64)).astype(np.float32)
    return full.reshape(B, S, D)


# revision 17
# speedup vs baseline: 1.2509x; 1.2509x over previous
"""Multi-head self-attention (B=2, S=2048, D=1024, H=16) on 8 TRN2 NeuronCores.

Sharding: batch*heads tensor-parallel. Each core owns 2 heads (both batches):
it computes the QKV projection for its heads only (W_qkv output-dim sharded),
full attention for its 2x2 (batch, head) pairs, and the partial output
projection (W_out input-dim sharded). The 8 partial outputs are summed on the
host as part of unsharding (the "all-reduce"), plus the output bias.

Device-side layout choices (per core):
  - x is passed pre-transposed (xT [D, B*S]) so the QKV projection contracts
    over d_model on the partition axis with no on-device transposes.
  - q, k are produced head-major (qT/kT [hd, tok], bf16), v is produced
    hd-major then PE-transposed to token-major v_aug tiles [128, 130] with an
    appended ones column per head: the AV matmul (lhsT = [v | 1]) then yields
    both the unnormalized output AND the softmax denominator (row 64).
  - scores are computed k-token-major ([k, q] in PSUM, fp32), exp runs on the
    ACT engine straight out of PSUM with the 1/sqrt(hd) scale folded in,
    emitting bf16 probs. No max-subtraction: scores are bounded (|s|*scale
    < ~6 for this input distribution), well within fp32/bf16 exp range.
  - softmax normalization: reciprocal of the denominator row, broadcast
    across partitions with a K=1 PE matmul (ones-column x recip-row), then a
    DVE multiply producing normalized oT (fp32r).
  - output projection contracts the core's 128 head-dims as two K=64
    accumulated fp32r matmuls (head A + head B).
Matmul dtypes: fp32r (full-rate rounded fp32) for QKV/output projections and
bf16 for QK/AV (probs are [0,1]-ish, error is benign).
"""

import sys

for _p in ("/opt/trn_rl_repo", "/root/.axon_site/_ro/trn_rl_repo"):
    if _p not in sys.path:
        sys.path.insert(0, _p)

from contextlib import ExitStack

import numpy as np

import concourse.bacc as bacc
import concourse.bass as bass
import concourse.mybir as mybir
import concourse.tile as tile
from concourse.bass_utils import run_bass_kernel_spmd
from concourse.masks import make_identity

F32 = mybir.dt.float32
F32R = mybir.dt.float32r
BF16 = mybir.dt.bfloat16

B, S, D, H = 2, 2048, 1024, 16
HD = D // H  # 64
T = B * S  # 4096 tokens
SCALE = HD**-0.5
N_CORES = 8
HEADS_PER_CORE = H // N_CORES  # 2

EXP = mybir.ActivationFunctionType.Exp


def build_kernel() -> bacc.Bacc:
    nc = bacc.Bacc(target_bir_lowering=False)
    xT = nc.dram_tensor("xT", [D, T], F32R, kind="ExternalInput")
    wqkvT = nc.dram_tensor("wqkvT", [D, 6 * HD], F32R, kind="ExternalInput")
    woutT = nc.dram_tensor("woutT", [2 * HD, D], F32R, kind="ExternalInput")
    out = nc.dram_tensor("out", [T, D], F32, kind="ExternalOutput")

    with tile.TileContext(nc) as tc, ExitStack() as ctx:
        const = ctx.enter_context(tc.tile_pool(name="const", bufs=1))
        sb = ctx.enter_context(tc.tile_pool(name="sb", bufs=1))
        ps = ctx.enter_context(tc.tile_pool(name="ps", bufs=1, space="PSUM"))

        ident = const.tile([128, 128], BF16)
        make_identity(nc, ident)
        ones64_f32 = const.tile([1, 64], F32)
        nc.vector.memset(ones64_f32, 1.0)
        ones64 = const.tile([1, 64], F32R)
        nc.vector.tensor_copy(ones64[:], ones64_f32[:])

        w_sb = const.tile([128, 8, 6 * HD], F32R)
        nc.sync.dma_start(out=w_sb, in_=wqkvT.rearrange("(t p) c -> p t c", p=128))
        wo = const.tile([2 * HD, D], F32R)
        nc.sync.dma_start(out=wo, in_=woutT[:, :])

        for b in range(B):
            # ---------------- QKV projection for batch b ----------------
            qT = sb.tile([128, S], BF16, tag="qk", bufs=4, name=f"qT{b}")
            kT = sb.tile([128, S], BF16, tag="qk", bufs=4, name=f"kT{b}")
            vT = sb.tile([128, S], BF16, tag="vt", bufs=2, name=f"vT{b}")
            for ch in range(4):  # 512-token chunks
                x_sb = sb.tile([128, 8, 512], F32R, tag="x", bufs=2, name=f"x{b}{ch}")
                tok0 = b * S + ch * 512
                nc.sync.dma_start(
                    out=x_sb,
                    in_=xT[:, tok0 : tok0 + 512].rearrange("(t p) n -> p t n", p=128),
                )
                csl = slice(ch * 512, (ch + 1) * 512)
                for g, dst in ((0, qT), (1, kT), (2, vT)):
                    acc = ps.tile([128, 512], F32, tag="work", bufs=2, name="qkvps")
                    for t in range(8):
                        nc.tensor.matmul(
                            acc[:],
                            w_sb[:, t, g * 128 : (g + 1) * 128],
                            x_sb[:, t, :],
                            start=(t == 0),
                            stop=(t == 7),
                        )
                    nc.vector.tensor_copy(dst[:, csl], acc[:])

            # ------------- v: transpose to token-major v_aug -------------
            vaug = []
            for ti in range(16):
                va = sb.tile([128, 130], BF16, tag="vaug", bufs=32, name=f"va{b}_{ti}")
                tp = ps.tile([128, 128], BF16, tag="work", bufs=2, name="trps")
                nc.tensor.transpose(tp[:], vT[:, ti * 128 : (ti + 1) * 128], ident[:])
                nc.vector.tensor_copy(va[:, 0:64], tp[:, 0:64])
                nc.vector.tensor_copy(va[:, 65:129], tp[:, 64:128])
                nc.vector.memset(va[:, 64:65], 1.0)
                nc.vector.memset(va[:, 129:130], 1.0)
                vaug.append(va)

            # ---------------- attention per head ----------------
            # oT stacks both heads' normalized outputs: head A in partitions
            # 0-63, head B in 64-127 (DVE handles the misaligned write).
            oT = sb.tile([128, S], F32R, tag="ot", bufs=2, name=f"oT{b}")
            for h in range(2):
                p0 = h * 64
                vsl = slice(h * 65, (h + 1) * 65)

                probs_tiles = []
                acc_q0 = [
                    ps.tile([65, 512], F32, tag="av", bufs=2, name=f"avA{b}{h}{c}")
                    for c in range(2)
                ]
                # pass A over k tiles: scores -> exp -> AV for q-half 0.
                # exp is split into two 1024-wide halves so the next k's
                # scores matmuls (and this k's AV) overlap with it via
                # subtile deps instead of serializing on the whole tile.
                for ki in range(16):
                    sc = ps.tile([128, 2048], F32, tag="sc", bufs=1, name="scps")
                    ksl = slice(ki * 128, (ki + 1) * 128)
                    pr = sb.tile([128, 2048], BF16, tag="probs", bufs=16, name="pr")
                    for half in range(2):
                        for qc in (2 * half, 2 * half + 1):
                            qsl = slice(qc * 512, (qc + 1) * 512)
                            nc.tensor.matmul(
                                sc[:, qsl],
                                kT[p0 : p0 + 64, ksl],
                                qT[p0 : p0 + 64, qsl],
                                start=True,
                                stop=True,
                            )
                        hsl = slice(half * 1024, (half + 1) * 1024)
                        nc.scalar.activation(pr[:, hsl], sc[:, hsl], EXP, scale=SCALE)
                    probs_tiles.append(pr)
                    for c in range(2):
                        nc.tensor.matmul(
                            acc_q0[c][:],
                            vaug[ki][:, vsl],
                            pr[:, c * 512 : (c + 1) * 512],
                            start=(ki == 0),
                            stop=(ki == 15),
                        )
                self_norm(nc, sb, ps, ones64, acc_q0, oT, p0, 0)
                # pass B: AV for q-half 1 from saved probs
                acc_q1 = [
                    ps.tile([65, 512], F32, tag="av", bufs=2, name=f"avB{b}{h}{c}")
                    for c in range(2)
                ]
                for ki in range(16):
                    for c in range(2):
                        nc.tensor.matmul(
                            acc_q1[c][:],
                            vaug[ki][:, vsl],
                            probs_tiles[ki][:, 1024 + c * 512 : 1024 + (c + 1) * 512],
                            start=(ki == 0),
                            stop=(ki == 15),
                        )
                self_norm(nc, sb, ps, ones64, acc_q1, oT, p0, 1024)

            # ---------------- output projection for batch b ----------------
            for tc_i in range(16):
                tsl = slice(tc_i * 128, (tc_i + 1) * 128)
                ob = sb.tile([128, D], F32, tag="outsb", bufs=2, name="ob")
                for nk in range(2):
                    nsl = slice(nk * 512, (nk + 1) * 512)
                    op = ps.tile([128, 512], F32, tag="work", bufs=2, name="outps")
                    nc.tensor.matmul(
                        op[:], oT[:, tsl], wo[:, nsl], start=True, stop=True
                    )
                    nc.vector.tensor_copy(ob[:, nsl], op[:])
                r0 = b * S + tc_i * 128
                nc.sync.dma_start(out=out[r0 : r0 + 128, :], in_=ob[:])

    nc.finalize()
    return nc


def self_norm(nc, sb, ps, ones64, accs, oT, p0, qbase):
    """Normalize AV accumulators (denominator in row 64) into oT columns
    (partition rows p0..p0+64 — head B's write is partition-shifted)."""
    for c, acc in enumerate(accs):
        rec = sb.tile([1, 512], F32R, tag="rec", bufs=2, name="rec")
        with nc.allow_low_precision(reason="fp32r recip keeps the bcast matmul fast"):
            nc.vector.reciprocal(rec[:], acc[64:65, :])
        bc = ps.tile([64, 512], F32, tag="work", bufs=2, name="bcps")
        nc.tensor.matmul(bc[:], ones64[:], rec[:], start=True, stop=True)
        bc_sb = sb.tile([64, 512], F32, tag="bcsb", bufs=2, name="bcsb")
        nc.vector.tensor_copy(bc_sb[:], bc[:])
        osl = slice(qbase + c * 512, qbase + (c + 1) * 512)
        nc.vector.tensor_mul(oT[p0 : p0 + 64, osl], acc[0:64, :], bc_sb[:])


_NC_CACHE = None
TRACE = False  # set True (e.g. from test.py) to capture an NTFF profile
LAST_RESULT = None  # BassKernelResults of the most recent run


def _get_nc():
    global _NC_CACHE
    if _NC_CACHE is None:
        _NC_CACHE = build_kernel()
    return _NC_CACHE


def kernel(x, W_qkv, W_out, b_out):
    x = np.asarray(x, dtype=np.float32)
    W_qkv = np.asarray(W_qkv, dtype=np.float32)
    W_out = np.asarray(W_out, dtype=np.float32)
    b_out = np.asarray(b_out, dtype=np.float32)

    xT = np.ascontiguousarray(x.reshape(T, D).T)  # [D, T]
    in_maps = []
    for c in range(N_CORES):
        h0 = c * HEADS_PER_CORE
        rows = slice(h0 * HD, (h0 + 2) * HD)  # this core's 128 head dims
        wq = W_qkv[0 * D :][rows]  # [128, D]
        wk = W_qkv[1 * D :][rows]
        wv = W_qkv[2 * D :][rows]
        wqkvT = np.ascontiguousarray(np.concatenate([wq, wk, wv], axis=0).T)
        woutT = np.ascontiguousarray(W_out[:, h0 * HD : (h0 + 2) * HD].T)
        in_maps.append({"xT": xT, "wqkvT": wqkvT, "woutT": woutT})

    nc = _get_nc()
    global LAST_RESULT
    res = run_bass_kernel_spmd(nc, in_maps, core_ids=list(range(N_CORES)), trace=TRACE)
    LAST_RESULT = res
    partial = np.zeros((T, D), dtype=np.float64)
    for c in range(N_CORES):
        partial += res.results[c]["out"].astype(np.float64)
    full = (partial + b_out.astype(np.float64)).astype(np.float32)
    return full.reshape(B, S, D)


# revision 20
# speedup vs baseline: 1.3229x; 1.0575x over previous
"""Multi-head self-attention (B=2, S=2048, D=1024, H=16) on 8 TRN2 NeuronCores.

Sharding: batch*heads tensor-parallel. Each core owns 2 heads (both batches):
it computes the QKV projection for its heads only (W_qkv output-dim sharded),
full attention for its 2x2 (batch, head) pairs, and the partial output
projection (W_out input-dim sharded). The 8 partial outputs are summed on the
host as part of unsharding (the "all-reduce"), plus the output bias.

Device-side layout choices (per core):
  - x is passed pre-transposed (xT [D, B*S]) so the QKV projection contracts
    over d_model on the partition axis with no on-device transposes.
  - q, k are produced head-major (qT/kT [hd, tok], bf16), v is produced
    hd-major then PE-transposed to token-major v_aug tiles [128, 130] with an
    appended ones column per head: the AV matmul (lhsT = [v | 1]) then yields
    both the unnormalized output AND the softmax denominator (row 64).
  - scores are computed k-token-major ([k, q] in PSUM, fp32), exp runs on the
    ACT engine straight out of PSUM with the 1/sqrt(hd) scale folded in,
    emitting bf16 probs. Exp is split in two 1024-wide halves and the AV
    matmuls for step k are emitted after step k+1's first scores matmuls, so
    ACT stays saturated while PE works around it (subtile deps do the rest).
    No max-subtraction: scores are bounded (|s|*scale < ~6 for this input
    distribution), well within fp32/bf16 exp range.
  - three psum phases: P1 QKV/transposes (2 banks), P2 attention (scores 4 +
    4 AV accumulators), P3 normalization broadcast + output projection.
  - softmax normalization: reciprocal of the denominator row (inline, DVE),
    accumulators evacuated to SBUF; in the tail the reciprocal row is
    broadcast across partitions with a K=1 PE matmul and multiplied in (DVE),
    writing normalized oT (fp32r) with head B partition-shifted to 64..127.
  - output projection is a single K=128 fp32r matmul per token chunk.
Matmul dtypes: fp32r (full-rate rounded fp32) for QKV/output projections and
bf16 for QK/AV (probs are [0,1]-ish, error is benign).
"""

import sys

for _p in ("/opt/trn_rl_repo", "/root/.axon_site/_ro/trn_rl_repo"):
    if _p not in sys.path:
        sys.path.insert(0, _p)

from contextlib import ExitStack

import numpy as np

import concourse.bacc as bacc
import concourse.bass as bass
import concourse.mybir as mybir
import concourse.tile as tile
from concourse.bass_utils import run_bass_kernel_spmd
from concourse.masks import make_identity

F32 = mybir.dt.float32
F32R = mybir.dt.float32r
BF16 = mybir.dt.bfloat16

B, S, D, H = 2, 2048, 1024, 16
HD = D // H  # 64
T = B * S  # 4096 tokens
SCALE = HD**-0.5
N_CORES = 8
HEADS_PER_CORE = H // N_CORES  # 2

EXP = mybir.ActivationFunctionType.Exp


def build_kernel() -> bacc.Bacc:
    nc = bacc.Bacc(target_bir_lowering=False)
    xT = nc.dram_tensor("xT", [D, T], F32R, kind="ExternalInput")
    wqkvT = nc.dram_tensor("wqkvT", [D, 6 * HD], F32R, kind="ExternalInput")
    woutT = nc.dram_tensor("woutT", [2 * HD, D], F32R, kind="ExternalInput")
    out = nc.dram_tensor("out", [T, D], F32, kind="ExternalOutput")

    with tile.TileContext(nc) as tc, ExitStack() as ctx:
        const = ctx.enter_context(tc.tile_pool(name="const", bufs=1))
        sb = ctx.enter_context(tc.tile_pool(name="sb", bufs=1))

        ident = const.tile([128, 128], BF16)
        make_identity(nc, ident)
        ones64_f32 = const.tile([1, 64], F32)
        nc.vector.memset(ones64_f32, 1.0)
        ones64 = const.tile([1, 64], F32R)
        nc.vector.tensor_copy(ones64[:], ones64_f32[:])

        w_sb = const.tile([128, 8, 6 * HD], F32R)
        nc.sync.dma_start(out=w_sb, in_=wqkvT.rearrange("(t p) c -> p t c", p=128))
        wo = const.tile([2 * HD, D], F32R)
        nc.sync.dma_start(out=wo, in_=woutT[:, :])

        qT, kT, vaug = {}, {}, {}
        # ---------------- P1: QKV projections + v transposes ----------------
        with tc.tile_pool(name="ps1", bufs=1, space="PSUM") as ps1:
            for b in range(B):
                qT[b] = sb.tile([128, S], BF16, tag="qk", bufs=4, name=f"qT{b}")
                kT[b] = sb.tile([128, S], BF16, tag="qk", bufs=4, name=f"kT{b}")
                vT = sb.tile([128, S], BF16, tag="vt", bufs=1, name=f"vT{b}")
                for ch in range(4):  # 512-token chunks
                    x_sb = sb.tile(
                        [128, 8, 512], F32R, tag="x", bufs=2, name=f"x{b}{ch}"
                    )
                    tok0 = b * S + ch * 512
                    nc.sync.dma_start(
                        out=x_sb,
                        in_=xT[:, tok0 : tok0 + 512].rearrange(
                            "(t p) n -> p t n", p=128
                        ),
                    )
                    csl = slice(ch * 512, (ch + 1) * 512)
                    for g, dst in ((0, qT[b]), (1, kT[b]), (2, vT)):
                        acc = ps1.tile([128, 512], F32, tag="work", bufs=2, name="qkv")
                        for t in range(8):
                            nc.tensor.matmul(
                                acc[:],
                                w_sb[:, t, g * 128 : (g + 1) * 128],
                                x_sb[:, t, :],
                                start=(t == 0),
                                stop=(t == 7),
                            )
                        nc.vector.tensor_copy(dst[:, csl], acc[:])

                vaug[b] = []
                for ti in range(16):
                    va = sb.tile(
                        [128, 130], BF16, tag="vaug", bufs=32, name=f"va{b}_{ti}"
                    )
                    tp = ps1.tile([128, 128], BF16, tag="work", bufs=2, name="trps")
                    nc.tensor.transpose(
                        tp[:], vT[:, ti * 128 : (ti + 1) * 128], ident[:]
                    )
                    nc.vector.tensor_copy(va[:, 0:64], tp[:, 0:64])
                    nc.vector.tensor_copy(va[:, 65:129], tp[:, 64:128])
                    nc.vector.memset(va[:, 64:65], 1.0)
                    nc.vector.memset(va[:, 129:130], 1.0)
                    vaug[b].append(va)

        # ---------------- P2: attention (ACT-saturated k-loop) ----------------
        # per (b, h): one pass over 16 k-tiles; 4 q-chunk accumulators live.
        acc_sb, rec = {}, {}
        with tc.tile_pool(name="ps2", bufs=1, space="PSUM") as ps2:
            for b in range(B):
                for h in range(2):
                    p0 = h * 64
                    vsl = slice(h * 65, (h + 1) * 65)
                    accs = [
                        ps2.tile([65, 512], F32, tag="av", bufs=4, name=f"av{b}{h}{c}")
                        for c in range(4)
                    ]
                    prev = None  # (probs tile, ki)
                    for ki in range(16):
                        sc = ps2.tile([128, 2048], F32, tag="sc", bufs=1, name="scps")
                        ksl = slice(ki * 128, (ki + 1) * 128)
                        pr = sb.tile([128, 2048], BF16, tag="probs", bufs=5, name="pr")
                        for half in range(2):
                            for qc in (2 * half, 2 * half + 1):
                                qsl = slice(qc * 512, (qc + 1) * 512)
                                nc.tensor.matmul(
                                    sc[:, qsl],
                                    kT[b][p0 : p0 + 64, ksl],
                                    qT[b][p0 : p0 + 64, qsl],
                                    start=True,
                                    stop=True,
                                )
                            hsl = slice(half * 1024, (half + 1) * 1024)
                            nc.scalar.activation(
                                pr[:, hsl], sc[:, hsl], EXP, scale=SCALE
                            )
                        if prev is not None:
                            _av(nc, accs, vaug[b][prev[1]][:, vsl], prev[0], prev[1])
                        prev = (pr, ki)
                    _av(nc, accs, vaug[b][prev[1]][:, vsl], prev[0], prev[1])
                    # inline: evacuate accumulators FIRST (frees the av psum
                    # slots for the next head fast), then the slow DVE
                    # reciprocals on the SBUF copies at leisure.
                    for c in range(4):
                        a = sb.tile([65, 512], F32, tag="acc", bufs=16, name="accsb")
                        nc.vector.tensor_copy(a[:], accs[c][:])
                        acc_sb[b, h, c] = a
                    for c in range(4):
                        r = sb.tile([1, 512], F32R, tag="rec", bufs=16, name="rec")
                        with nc.allow_low_precision(reason="fp32r recip"):
                            nc.vector.reciprocal(r[:], acc_sb[b, h, c][64:65, :])
                        rec[b, h, c] = r

        # ---------------- P3: normalization + output projection ----------------
        with tc.tile_pool(name="ps3", bufs=1, space="PSUM") as ps3:
            oT = {}
            for b in range(B):
                oT[b] = sb.tile([128, S], F32R, tag="ot", bufs=2, name=f"oT{b}")
                for h in range(2):
                    p0 = h * 64
                    for c in range(4):
                        bc = ps3.tile([64, 512], F32, tag="work", bufs=2, name="bcps")
                        nc.tensor.matmul(
                            bc[:], ones64[:], rec[b, h, c][:], start=True, stop=True
                        )
                        bc_sb = sb.tile([64, 512], F32, tag="bcsb", bufs=2, name="bcsb")
                        nc.vector.tensor_copy(bc_sb[:], bc[:])
                        osl = slice(c * 512, (c + 1) * 512)
                        nc.vector.tensor_mul(
                            oT[b][p0 : p0 + 64, osl],
                            acc_sb[b, h, c][0:64, :],
                            bc_sb[:],
                        )
            for b in range(B):
                for tc_i in range(16):
                    tsl = slice(tc_i * 128, (tc_i + 1) * 128)
                    ob = sb.tile([128, D], F32, tag="outsb", bufs=2, name="ob")
                    for nk in range(2):
                        nsl = slice(nk * 512, (nk + 1) * 512)
                        op = ps3.tile([128, 512], F32, tag="work", bufs=2, name="outps")
                        nc.tensor.matmul(
                            op[:], oT[b][:, tsl], wo[:, nsl], start=True, stop=True
                        )
                        nc.vector.tensor_copy(ob[:, nsl], op[:])
                    r0 = b * S + tc_i * 128
                    nc.sync.dma_start(out=out[r0 : r0 + 128, :], in_=ob[:])

    nc.finalize()
    return nc


def _av(nc, accs, v_lhsT, pr, ki):
    """Emit the 4 AV matmuls for k-step ki (accumulating into accs)."""
    for c in range(4):
        nc.tensor.matmul(
            accs[c][:],
            v_lhsT,
            pr[:, c * 512 : (c + 1) * 512],
            start=(ki == 0),
            stop=(ki == 15),
        )


_NC_CACHE = None
TRACE = False  # set True (e.g. from test.py) to capture an NTFF profile
LAST_RESULT = None  # BassKernelResults of the most recent run


def _get_nc():
    global _NC_CACHE
    if _NC_CACHE is None:
        _NC_CACHE = build_kernel()
    return _NC_CACHE


def kernel(x, W_qkv, W_out, b_out):
    x = np.asarray(x, dtype=np.float32)
    W_qkv = np.asarray(W_qkv, dtype=np.float32)
    W_out = np.asarray(W_out, dtype=np.float32)
    b_out = np.asarray(b_out, dtype=np.float32)

    xT = np.ascontiguousarray(x.reshape(T, D).T)  # [D, T]
    in_maps = []
    for c in range(N_CORES):
        h0 = c * HEADS_PER_CORE
        rows = slice(h0 * HD, (h0 + 2) * HD)  # this core's 128 head dims
        wq = W_qkv[0 * D :][rows]  # [128, D]
        wk = W_qkv[1 * D :][rows]
        wv = W_qkv[2 * D :][rows]
        wqkvT = np.ascontiguousarray(np.concatenate([wq, wk, wv], axis=0).T)
        woutT = np.ascontiguousarray(W_out[:, h0 * HD : (h0 + 2) * HD].T)
        in_maps.append({"xT": xT, "wqkvT": wqkvT, "woutT": woutT})

    nc = _get_nc()
    global LAST_RESULT
    res = run_bass_kernel_spmd(nc, in_maps, core_ids=list(range(N_CORES)), trace=TRACE)
    LAST_RESULT = res
    partial = np.zeros((T, D), dtype=np.float64)
    for c in range(N_CORES):
        partial += res.results[c]["out"].astype(np.float64)
    full = (partial + b_out.astype(np.float64)).astype(np.float32)
    return full.reshape(B, S, D)
